# revision 10
# baseline (speedup 1.0000x reference)
"""Trainium2 Bass kernel for nn_PathSampling.

Computes, for each start node: masked path-node ids, per-path centrality
scores (sum over path positions), top-k_path=8 paths of 32 by score, and
returns the selected masked paths + edge-id rows.

Sharding: n_node (100000) split contiguously across 8 NeuronCores (12500
each, padded to 12544 = 98*128 on host); centrality table replicated.

Per-core pipeline (tiles of 128 nodes, one node per SBUF partition):
  1. DMA in paths tile [128, 32*8] + rand_lens tile [128, 32].
  2. DVE: mask = (j <= rand_len); masked paths mp (-1 fill) and gather
     index tile gidx (OOB-BIG fill, so masked slots skip the DMA gather).
  3. SWDGE indirect DMA gather of centrality[gidx] with bounds_check ->
     masked slots stay memset 0.0.
  4. DVE tensor_reduce(add) over the 8 path positions -> scores [128,32].
  5. DVE hardware top-8: max + max_index (tie semantics == lax.top_k).
  6. Masked paths tile written to an internal DRAM buffer; row offsets
     (node*32 + topk_idx) computed on DVE.
  7. SWDGE indirect row gathers: masked-path rows [8 i32] and edge rows
     [7 i32] straight from DRAM.
  8. DMA out selected rows.
"""

import os
import sys

import numpy as np

for _p in ("/opt/trn_rl_repo", "/root/.axon_site/_ro/trn_rl_repo"):
    if os.path.isdir(_p) and _p not in sys.path:
        sys.path.insert(0, _p)

import concourse.bacc as bacc
import concourse.bass as bass
import concourse.mybir as mybir
import concourse.tile as tile
from concourse import bass_utils

N_GRAPH = 100000
N_PATH = 32
L_PATH = 8
K_PATH = 8
N_CORES = 8
N_NODE = 100000
NODES_PER_CORE = 12500
TILE_P = 128
BIG = 1 << 20  # OOB marker for masked gather slots


def build_program(n_nodes: int):
    """One-core program; SPMD-replicated across cores with different inputs."""
    assert n_nodes % TILE_P == 0
    n_tiles = n_nodes // TILE_P

    nc = bacc.Bacc(
        "TRN2", target_bir_lowering=False, debug=False, enable_asserts=False
    )
    paths = nc.dram_tensor(
        "paths", [n_nodes, N_PATH, L_PATH], mybir.dt.int32, kind="ExternalInput"
    ).ap()
    edges = nc.dram_tensor(
        "edges", [n_nodes, N_PATH, L_PATH - 1], mybir.dt.int32, kind="ExternalInput"
    ).ap()
    rand = nc.dram_tensor(
        "rand", [n_nodes, N_PATH], mybir.dt.int32, kind="ExternalInput"
    ).ap()
    cent = nc.dram_tensor(
        "cent", [N_GRAPH, 1], mybir.dt.float32, kind="ExternalInput"
    ).ap()
    psel = nc.dram_tensor(
        "psel", [n_nodes, K_PATH, L_PATH], mybir.dt.int32, kind="ExternalOutput"
    ).ap()
    esel = nc.dram_tensor(
        "esel", [n_nodes, K_PATH, L_PATH - 1], mybir.dt.int32, kind="ExternalOutput"
    ).ap()
    mpaths = nc.dram_tensor(
        "mpaths", [n_nodes, N_PATH, L_PATH], mybir.dt.int32, kind="Internal"
    ).ap()

    paths2 = paths.rearrange("n a b -> n (a b)")  # [n, 256]
    mpaths2 = mpaths.rearrange("n a b -> n (a b)")
    mpaths_rows = mpaths.rearrange("n a b -> (n a) b")  # [n*32, 8]
    edges_rows = edges.rearrange("n a b -> (n a) b")  # [n*32, 7]
    psel2 = psel.rearrange("n k l -> n (k l)")  # [n, 64]
    esel2 = esel.rearrange("n k l -> n (k l)")  # [n, 56]

    F = N_PATH * L_PATH  # 256

    with tile.TileContext(nc) as tc:
        with (
            tc.tile_pool(name="consts", bufs=1) as cpool,
            tc.tile_pool(name="work", bufs=3) as pool,
        ):
            # jpos[p, (a b)] = b  (path position 0..7, repeating)
            jpos = cpool.tile([TILE_P, F], mybir.dt.int32)
            nc.gpsimd.iota(
                jpos[:], pattern=[[0, N_PATH], [1, L_PATH]], base=0, channel_multiplier=0
            )
            # pio[p, 0] = p * 32  (row offset of this partition's node)
            pio = cpool.tile([TILE_P, 1], mybir.dt.int32)
            nc.gpsimd.iota(pio[:], pattern=[[0, 1]], base=0, channel_multiplier=N_PATH)
            bc_reg = nc.gpsimd.to_reg(N_GRAPH - 1)

            for t in range(n_tiles):
                sl = slice(t * TILE_P, (t + 1) * TILE_P)

                pt = pool.tile([TILE_P, F], mybir.dt.int32)
                nc.sync.dma_start(pt[:], paths2[sl])
                rt = pool.tile([TILE_P, N_PATH], mybir.dt.int32)
                nc.sync.dma_start(rt[:], rand[sl])

                # keep (j <= rand_len): mask = (rand_bcast >= jpos)
                m = pool.tile([TILE_P, F], mybir.dt.int32)
                rt_b = rt[:].unsqueeze(2).to_broadcast([TILE_P, N_PATH, L_PATH])
                nc.vector.tensor_tensor(
                    out=m[:].rearrange("p (a b) -> p a b", b=L_PATH),
                    in0=rt_b,
                    in1=jpos[:].rearrange("p (a b) -> p a b", b=L_PATH),
                    op=mybir.AluOpType.is_ge,
                )

                mp = pool.tile([TILE_P, F], mybir.dt.int32)
                nc.vector.memset(mp[:], -1)
                nc.vector.copy_predicated(mp[:], m[:], pt[:])

                gidx = pool.tile([TILE_P, F], mybir.dt.int32)
                nc.vector.memset(gidx[:], BIG)
                nc.vector.copy_predicated(gidx[:], m[:], pt[:])

                gv = pool.tile([TILE_P, F], mybir.dt.float32)
                nc.vector.memset(gv[:], 0.0)
                # HW indirect DMA consumes ONE offset per output partition row
                # (gathering a contiguous run) -> one instruction per column.
                for j in range(F):
                    nc.gpsimd.indirect_dma_start(
                        out=gv[:, j : j + 1],
                        out_offset=None,
                        in_=cent,
                        in_offset=bass.IndirectOffsetOnAxis(ap=gidx[:, j : j + 1], axis=0),
                        bounds_check=bc_reg,
                        oob_is_err=False,
                    )

                s = pool.tile([TILE_P, N_PATH], mybir.dt.float32)
                nc.vector.tensor_reduce(
                    out=s[:],
                    in_=gv[:].rearrange("p (a b) -> p a b", b=L_PATH),
                    axis=mybir.AxisListType.X,
                    op=mybir.AluOpType.add,
                )

                mx = pool.tile([TILE_P, K_PATH], mybir.dt.float32)
                ix = pool.tile([TILE_P, K_PATH], mybir.dt.uint32)
                nc.vector.max(mx[:], s[:])
                nc.vector.max_index(ix[:], mx[:], s[:])

                # off = topk_idx + node*32
                off = pool.tile([TILE_P, K_PATH], mybir.dt.int32)
                nc.vector.scalar_tensor_tensor(
                    out=off[:],
                    in0=ix[:],
                    scalar=t * TILE_P * N_PATH,
                    op0=mybir.AluOpType.add,
                    in1=pio[:].to_broadcast([TILE_P, K_PATH]),
                    op1=mybir.AluOpType.add,
                )

                nc.sync.dma_start(mpaths2[sl], mp[:])

                ps = pool.tile([TILE_P, K_PATH * L_PATH], mybir.dt.int32)
                es = pool.tile([TILE_P, K_PATH * (L_PATH - 1)], mybir.dt.int32)
                for k in range(K_PATH):
                    nc.gpsimd.indirect_dma_start(
                        out=ps[:, k * L_PATH : (k + 1) * L_PATH],
                        out_offset=None,
                        in_=mpaths_rows,
                        in_offset=bass.IndirectOffsetOnAxis(ap=off[:, k : k + 1], axis=0),
                    )
                    nc.gpsimd.indirect_dma_start(
                        out=es[:, k * (L_PATH - 1) : (k + 1) * (L_PATH - 1)],
                        out_offset=None,
                        in_=edges_rows,
                        in_offset=bass.IndirectOffsetOnAxis(ap=off[:, k : k + 1], axis=0),
                    )

                nc.sync.dma_start(psel2[sl], ps[:])
                nc.sync.dma_start(esel2[sl], es[:])

    nc.compile()
    return nc


_PROGRAM_CACHE: dict[int, "bass.Bass"] = {}
LAST_RESULTS = None


def get_program(n_nodes: int):
    if n_nodes not in _PROGRAM_CACHE:
        _PROGRAM_CACHE[n_nodes] = build_program(n_nodes)
    return _PROGRAM_CACHE[n_nodes]


def kernel(paths, edge_ids, rand_lens, centrality, k_path):
    assert int(k_path) == K_PATH
    paths = np.asarray(paths, dtype=np.int32)
    edge_ids = np.asarray(edge_ids, dtype=np.int32)
    rand_lens = np.asarray(rand_lens, dtype=np.int32)
    centrality = np.asarray(centrality, dtype=np.float32)

    n_node = paths.shape[0]
    per_core = (n_node + N_CORES - 1) // N_CORES
    padded = ((per_core + TILE_P - 1) // TILE_P) * TILE_P

    cent_in = centrality.reshape(N_GRAPH, 1)

    in_maps = []
    for c in range(N_CORES):
        lo, hi = c * per_core, min((c + 1) * per_core, n_node)
        n_here = hi - lo
        pad = padded - n_here

        def shard(x):
            x = x[lo:hi]
            if pad:
                x = np.concatenate(
                    [x, np.zeros((pad,) + x.shape[1:], dtype=x.dtype)], axis=0
                )
            return np.ascontiguousarray(x)

        in_maps.append(
            {
                "paths": shard(paths),
                "edges": shard(edge_ids),
                "rand": shard(rand_lens),
                "cent": cent_in,
            }
        )

    nc = get_program(padded)
    res = bass_utils.run_bass_kernel_spmd(nc, in_maps, core_ids=list(range(N_CORES)))
    global LAST_RESULTS
    LAST_RESULTS = res

    psel_parts = []
    esel_parts = []
    for c in range(N_CORES):
        lo, hi = c * per_core, min((c + 1) * per_core, n_node)
        n_here = hi - lo
        psel_parts.append(res.results[c]["psel"][:n_here])
        esel_parts.append(res.results[c]["esel"][:n_here])

    paths_sel = np.concatenate(psel_parts, axis=0).astype(np.int32)
    edge_ids_sel = np.concatenate(esel_parts, axis=0).astype(np.int32)
    return paths_sel, edge_ids_sel
